# revision 26
# baseline (speedup 1.0000x reference)
"""GCN message-passing kernel for 8 Trainium2 NeuronCores (Bass/Tile).

Computation:  out = (segment_sum(relu(x@W1+b1)[edge_src], edge_dst)) @ W2 + b2

v4 "message-direct" design: destination nodes are partitioned across the 8
cores (degree-balanced, uniform tile template).  The HOST lays out x rows in
edge order (duplicating rows of x per edge, transposed, bf16) so each core
streams dense [128 features x 128 messages] tiles and computes
relu(x[src]@W1+b1) per MESSAGE with plain matmuls — the result lands directly
in the arena layout that the one-hot segment-sum matmuls consume.  No hidden
table, no SWDGE gather, no irregular device traffic at all: the irregular
access moves to host-side numpy fancy indexing (the same class of host prep
as the edge bucketing/transposes the kernel already does).

Per core: ~1600 message tiles -> 1600 message matmuls + relu (split between
Scalar and GpSimd engines), 1600 one-hot matmuls (PSUM-accumulated per dst
block), 98 W2 matmuls.  All DMA is streaming (xM 52MB/core, out 6.4MB/core).

Compute per (block, class) tiles follow a uniform template U (mostly 4 tiles
= 512 edges per group plus a few 5s) with a host vector-bin-packing pass, so
one compiled program serves all 8 cores; per-core data (xM, doff) differs.
"""

import os
import sys

sys.path.insert(0, "/opt/trn_rl_repo")

import numpy as np

import bass_rust
import concourse.bass as bass
import concourse.bacc as bacc
import concourse.mybir as mybir
import concourse.tile as tile_mod
from concourse.tile import TileContext
from concourse.bass_utils import run_bass_kernel_spmd
from concourse import library_config
from concourse._compat import cdiv

NCORES = 8
D = 128
P = 128
NQUART = 4           # src-range classes (kept for balanced template packing)
BATCH_BLOCKS = int(os.environ.get("GCN_BATCH_BLOCKS", "3"))

_PATCHED = False


def _patch_tile_drain():
    """This walrus build only accepts ONE sync-wait on a CTRL (Drain)
    instruction; Tile's end-of-kernel drain carries one wait per DMA sem
    lane.  Split the waits across multiple drain instructions."""
    global _PATCHED
    if _PATCHED:
        return
    _PATCHED = True

    def _patched_dab(self, tick_clock, wait_clock):
        nc = self.nc
        from concourse.vector_clock import ScopedClock

        drain_inst = nc.sync.drain()
        wait_clock.add_sem_waits(
            drain_inst.ins, ScopedClock({None: tick_clock.global_clock})
        )
        si = drain_inst.ins.sync_info
        if si is not None and si.on_wait is not None and len(si.on_wait) > 1:
            waits = list(si.on_wait)
            drain_inst.ins.sync_info = bass_rust.SyncInfo(
                on_wait=[waits[0]], on_update=list(si.on_update or [])
            )
            for w in waits[1:]:
                extra = nc.sync.drain()
                extra.ins.sync_info = bass_rust.SyncInfo(on_wait=[w], on_update=[])
        nc.all_engine_barrier()
        assert self.sems is not None
        popped = nc._tile_sem_poison_stack.pop()
        assert popped is self._sem_poison
        nc.clear_and_free_semaphores(list(self.sems.allocated().values()))
        nc.all_engine_barrier()

    tile_mod.TileContext._drain_and_barrier = _patched_dab


def _assign_nodes(deg, n_parts, part_capacity):
    order = np.argsort(-deg, kind="stable")
    part = np.empty(len(deg), np.int32)
    n = len(deg)
    fwd = np.arange(n_parts)
    rev = fwd[::-1]
    pos = 0
    row = 0
    while pos < n:
        chunk = order[pos : pos + n_parts]
        lane = fwd if (row % 2 == 0) else rev
        part[chunk] = lane[: len(chunk)]
        pos += n_parts
        row += 1
    counts = np.bincount(part, minlength=n_parts)
    assert counts.max() <= part_capacity, (counts.max(), part_capacity)
    return part


def _make_template(nblk, nf):
    U = np.full((nblk, NQUART), 4, np.int64)
    for r in range(NQUART):
        picks = (np.arange(nf) * nblk // max(nf, 1) + r * 3) % nblk
        U[picks, r] += 1
    return U


def _pack_core(nv, U, nblk):
    n = len(nv)
    cap = (U * P).astype(np.float64)
    order = np.argsort(-nv.sum(1), kind="stable")
    L = np.zeros((nblk, NQUART), np.int64)
    sz = np.zeros(nblk, np.int64)
    blk = np.full(n, -1, np.int32)
    for g in order:
        v = nv[g]
        ok = (sz < P) & np.all(L + v <= cap, axis=1)
        cand = np.nonzero(ok)[0]
        if len(cand) == 0:
            cand = np.nonzero(sz < P)[0]
            Lc = L[cand] + v
            over = np.maximum(Lc - cap[cand], 0).sum(1)
            b = cand[np.argmin(over * 100000 + Lc.max(1))]
        else:
            frac = (L[cand] + v) / cap[cand]
            score = frac.max(1) * 1000 + sz[cand] * 0.001
            b = cand[np.argmin(score)]
        blk[g] = b
        L[b] += v
        sz[b] += 1
    t = np.ceil(L / P).astype(np.int64)
    excess = int(np.maximum(t - U, 0).sum())
    return blk, t, excess


def _build_host_plan(x, edge_src, edge_dst, W1, b1, W2, b2):
    import ml_dtypes

    N, Dd = x.shape
    E = edge_src.shape[0]
    assert Dd == D
    nodes_per_core = cdiv(N, NCORES)
    nblk = cdiv(nodes_per_core, P)
    quart = cdiv(N, NQUART)

    e_cls_all = (edge_src // quart).astype(np.int64)
    nodecls = np.zeros((N, NQUART), np.int64)
    np.add.at(nodecls, (edge_dst, e_cls_all), 1)
    deg = nodecls.sum(1)

    core_of = _assign_nodes(deg, NCORES, nodes_per_core)

    core_nodes = [np.nonzero(core_of == c)[0] for c in range(NCORES)]
    U = None
    blks = None
    for nf in (8, 12, 18, 32, 49, 98):
        Utry = _make_template(nblk, nf)
        bl = []
        ok = True
        for c in range(NCORES):
            blk_c, t_c, excess = _pack_core(nodecls[core_nodes[c]], Utry, nblk)
            if excess > 0:
                ok = False
                break
            bl.append(blk_c)
        if ok:
            U = Utry
            blks = bl
            break
    if U is None:
        bl = []
        ts = []
        for c in range(NCORES):
            part = _assign_nodes(deg[core_nodes[c]], nblk, P)
            mask = core_of[edge_dst] == c
            pos = np.searchsorted(core_nodes[c], edge_dst[mask])
            L = np.zeros((nblk, NQUART), np.int64)
            np.add.at(L, (part[pos], e_cls_all[mask]), 1)
            bl.append(part)
            ts.append(np.ceil(L / P).astype(np.int64))
        U = np.max(np.stack(ts), axis=0)
        blks = bl

    blk_of = np.empty(N, np.int32)
    off_of = np.empty(N, np.int32)
    for c in range(NCORES):
        nodes_c = core_nodes[c]
        part = blks[c]
        for b in range(nblk):
            members = nodes_c[part == b]
            blk_of[members] = b
            off_of[members] = np.arange(len(members), dtype=np.int32)

    e_core = core_of[edge_dst]
    e_blk = blk_of[edge_dst]
    e_off = off_of[edge_dst]

    batches = []
    b0 = 0
    while b0 < nblk:
        batches.append(min(BATCH_BLOCKS, nblk - b0))
        b0 += BATCH_BLOCKS
    nbatch = len(batches)
    batch_of_blk = np.repeat(np.arange(nbatch), batches)

    NT_b = U.sum(1)
    NTmax = int(NT_b.max())
    ct = np.zeros((nbatch, NQUART), np.int64)
    for k in range(nbatch):
        blo = k * BATCH_BLOCKS
        ct[k] = U[blo : blo + batches[k]].sum(0)
    cstart = np.zeros((nbatch, NQUART + 1), np.int64)
    np.cumsum(ct, axis=1, out=cstart[:, 1:])
    TT = cstart[:, NQUART]
    TTmax = int(TT.max())

    key = (e_core.astype(np.int64) * nblk + e_blk) * NQUART + e_cls_all
    order = np.argsort(key, kind="stable")
    src_s = edge_src[order]
    off_s = e_off[order]
    key_s = key[order]
    gc = np.bincount(key, minlength=NCORES * nblk * NQUART)
    starts = np.zeros(NCORES * nblk * NQUART + 1, np.int64)
    np.cumsum(gc, out=starts[1:])
    within = np.arange(E, dtype=np.int64) - starts[key_s]

    c_ = key_s // (nblk * NQUART)
    b_ = (key_s // NQUART) % nblk
    r_ = key_s % NQUART
    k_ = batch_of_blk[b_]

    tile_in_grp = within // P
    lane = within % P
    assert np.all(tile_in_grp < U[b_, r_]), "packing exceeded template"

    Ucum_blk = np.cumsum(U, axis=0)
    blo_ = (k_ * BATCH_BLOCKS).astype(np.int64)
    prev_b = np.where(b_ > 0, Ucum_blk[b_ - 1, r_], 0)
    prev_b0 = np.where(blo_ > 0, Ucum_blk[blo_ - 1, r_], 0)
    tiles_before_in_class = prev_b - prev_b0
    arena_tile = cstart[k_, r_] + tiles_before_in_class + tile_in_grp
    slot = arena_tile * P + lane

    # message source node per arena slot (pad slots -> node 0, masked by doff=-1)
    src_of_slot = np.zeros((NCORES, nbatch, TTmax * P), np.int64)
    sflat = (c_ * nbatch + k_) * (TTmax * P) + slot
    src_of_slot.reshape(-1)[sflat] = src_s

    Ucum_cls = np.cumsum(U, axis=1)
    prev_cls = np.where(r_ > 0, Ucum_cls[b_, r_ - 1], 0)
    ohcol = prev_cls + tile_in_grp
    dstoff_all = np.full((NCORES, nblk, 128, NTmax), -1.0, np.float32)
    dflat = ((c_ * nblk + b_) * 128 + lane) * NTmax + ohcol
    dstoff_all.reshape(-1)[dflat] = off_s.astype(np.float32)

    acol = np.full((nblk, NTmax), -1, np.int64)
    for b in range(nblk):
        k = batch_of_blk[b]
        blo = k * BATCH_BLOCKS
        jj = 0
        for r in range(NQUART):
            before = int(U[blo:b, r].sum())
            base = int(cstart[k, r]) + before
            for i in range(int(U[b, r])):
                acol[b, jj] = base + i
                jj += 1
        assert jj == NT_b[b]

    # ---- xM: per-core per-batch transposed message-x tiles (bf16) ----
    x16 = np.asarray(x, np.float32).astype(ml_dtypes.bfloat16)
    xM = np.empty((NCORES, nbatch, P, TTmax * P), ml_dtypes.bfloat16)
    for c in range(NCORES):
        g = x16[src_of_slot[c].reshape(-1)]          # [nbatch*TT*P, 128]
        xM[c] = g.reshape(nbatch, TTmax * P, P).transpose(0, 2, 1)

    inv = np.zeros((NCORES, nblk * P), np.int64)
    nodes = np.arange(N, dtype=np.int64)
    inv_index = core_of.astype(np.int64) * (nblk * P) + blk_of * P + off_of
    inv.reshape(-1)[inv_index] = nodes
    valid = np.zeros((NCORES, nblk * P), bool)
    valid.reshape(-1)[inv_index] = True

    plan = dict(
        N=N, E=E, nblk=nblk, batches=batches, nbatch=nbatch,
        U=U, NT_b=NT_b, NTmax=NTmax, ct=ct, cstart=cstart, TTmax=TTmax,
        acol=acol, xM=xM, dstoff_all=dstoff_all,
        inv=inv, valid=valid,
        has_b1=bool(np.any(np.asarray(b1))), has_b2=bool(np.any(np.asarray(b2))),
    )
    return plan


def _build_program(plan):
    _patch_tile_drain()
    nblk = plan["nblk"]
    batches = plan["batches"]
    nbatch = plan["nbatch"]
    NTmax = plan["NTmax"]
    TTmax = plan["TTmax"]
    has_b1 = plan["has_b1"]
    has_b2 = plan["has_b2"]
    NT_b = plan["NT_b"]
    acol = plan["acol"]

    nc = bacc.Bacc("TRN2", debug=False)
    f32 = mybir.dt.float32
    bf16 = mybir.dt.bfloat16

    xM_t = nc.dram_tensor("xm", [nbatch, P, TTmax * P], bf16, kind="ExternalInput")
    doff_t = nc.dram_tensor("doff", [nblk, 128, NTmax], bf16, kind="ExternalInput")
    w1_t = nc.dram_tensor("w1", [P, P], bf16, kind="ExternalInput")
    w2_t = nc.dram_tensor("w2", [P, P], bf16, kind="ExternalInput")
    b1_t = nc.dram_tensor("b1", [1, P], bf16, kind="ExternalInput")
    b2_t = nc.dram_tensor("b2", [1, P], bf16, kind="ExternalInput")
    iota_t = nc.dram_tensor("iota", [P, NTmax * P], bf16, kind="ExternalInput")
    ones_t = nc.dram_tensor("ones", [1, P], bf16, kind="ExternalInput")
    out_t = nc.dram_tensor("out", [nblk * P, P], f32, kind="ExternalOutput")

    relu = mybir.ActivationFunctionType.Relu

    with TileContext(nc) as tc:
        nc.gpsimd.load_library(library_config.mlp)
        with (
            tc.tile_pool(name="const", bufs=1) as constp,
            tc.tile_pool(name="xm", bufs=3) as xmp,
            tc.tile_pool(name="p1", bufs=3, space="PSUM") as p1,
            tc.tile_pool(name="arena", bufs=3) as arenap,
            tc.tile_pool(name="dop", bufs=4) as dop,
            tc.tile_pool(name="ohp", bufs=4) as ohp,
            tc.tile_pool(name="p2", bufs=3, space="PSUM") as p2,
            tc.tile_pool(name="agp", bufs=3) as agp,
            tc.tile_pool(name="p3", bufs=2, space="PSUM") as p3,
            tc.tile_pool(name="outp", bufs=3) as outp,
        ):
            w1s = constp.tile([P, P], bf16, tag="w1")
            nc.sync.dma_start(w1s[:], w1_t[:])
            w2s = constp.tile([P, P], bf16, tag="w2")
            nc.sync.dma_start(w2s[:], w2_t[:])
            b1s = constp.tile([1, P], bf16, tag="b1")
            nc.sync.dma_start(b1s[:], b1_t[:])
            b2s = constp.tile([1, P], bf16, tag="b2")
            nc.sync.dma_start(b2s[:], b2_t[:])
            iotas = constp.tile([P, NTmax * P], bf16, tag="iota")
            nc.sync.dma_start(iotas[:], iota_t[:])
            oness = constp.tile([1, P], bf16, tag="ones")
            nc.sync.dma_start(oness[:], ones_t[:])

            blk0 = 0
            for k in range(nbatch):
                B = batches[k]
                TT = int(plan["cstart"][k][NQUART])
                xm = xmp.tile([P, TTmax * P], bf16, tag="xm")
                nc.sync.dma_start(xm[:, : TT * P], xM_t[k, :, : TT * P])
                ar = arenap.tile([128, TTmax, P], bf16, tag="ar")
                arv = ar[:].rearrange("p t f -> p (t f)")
                # message compute: groups of 4 tiles share one PSUM bank;
                # relu+cast alternates between Scalar and GpSimd engines
                ng = cdiv(TT, 4)
                for g in range(ng):
                    g0 = g * 4
                    gn = min(4, TT - g0)
                    ph = p1.tile([P, 4 * P], f32, tag="p1")
                    for i in range(gn):
                        t = g0 + i
                        if has_b1:
                            nc.tensor.matmul(ph[:, i * P : (i + 1) * P], xm[:, t * P : (t + 1) * P], w1s[:], start=True, stop=False)
                            nc.tensor.matmul(ph[:, i * P : (i + 1) * P], oness[:], b1s[:], start=False, stop=True)
                        else:
                            nc.tensor.matmul(ph[:, i * P : (i + 1) * P], xm[:, t * P : (t + 1) * P], w1s[:], start=True, stop=True)
                    dst = arv[:, g0 * P : (g0 + gn) * P]
                    nc.scalar.activation(dst, ph[:, : gn * P], relu)
                for j in range(B):
                    blk = blk0 + j
                    NT = int(NT_b[blk])
                    do = dop.tile([128, NTmax], bf16, tag="do")
                    nc.sync.dma_start(do[:], doff_t[blk])
                    oh = ohp.tile([P, NTmax, P], bf16, tag="oh")
                    nc.vector.tensor_tensor(
                        oh[:, :NT, :],
                        do[:, :NT].to_broadcast([P, NT, P]),
                        iotas[:, : NT * P].rearrange("p (j d) -> p j d", j=NT),
                        op=mybir.AluOpType.is_equal,
                    )
                    pa = p2.tile([P, P], f32, tag="p2")
                    for jj in range(NT):
                        col = int(acol[blk, jj])
                        nc.tensor.matmul(
                            pa[:], ar[:, col, :], oh[:, jj, :],
                            start=(jj == 0), stop=(jj == NT - 1),
                        )
                    ag = agp.tile([P, P], bf16, tag="ag")
                    nc.vector.tensor_copy(ag[:], pa[:])
                    po = p3.tile([P, P], f32, tag="p3")
                    if has_b2:
                        nc.tensor.matmul(po[:], ag[:], w2s[:], start=True, stop=False)
                        nc.tensor.matmul(po[:], oness[:], b2s[:], start=False, stop=True)
                    else:
                        nc.tensor.matmul(po[:], ag[:], w2s[:], start=True, stop=True)
                    ot = outp.tile([P, P], f32, tag="ot")
                    nc.vector.tensor_copy(ot[:], po[:])
                    nc.sync.dma_start(out_t[blk * P : (blk + 1) * P, :], ot[:])
                blk0 += B

    nc.compile()
    return nc


def _to_bf16(a):
    import ml_dtypes
    return np.asarray(a, np.float32).astype(ml_dtypes.bfloat16)


def _run(plan, W1, b1, W2, b2, trace=False):
    NTmax = plan["NTmax"]
    iota_rep = np.tile(
        np.arange(P, dtype=np.float32)[None, None, :], (P, NTmax, 1)
    ).reshape(P, NTmax * P)
    ones = np.ones((1, P), np.float32)
    in_maps = []
    for c in range(NCORES):
        in_maps.append({
            "xm": plan["xM"][c],
            "doff": _to_bf16(plan["dstoff_all"][c]),
            "w1": _to_bf16(np.asarray(W1).reshape(P, P)),
            "w2": _to_bf16(np.asarray(W2).reshape(P, P)),
            "b1": _to_bf16(np.asarray(b1).reshape(1, P)),
            "b2": _to_bf16(np.asarray(b2).reshape(1, P)),
            "iota": _to_bf16(iota_rep),
            "ones": _to_bf16(ones),
        })
    nc = _build_program(plan)
    res = run_bass_kernel_spmd(nc, in_maps, core_ids=list(range(NCORES)), trace=trace)
    return res


def kernel(x, edge_src, edge_dst, W1, b1, W2, b2, _trace=False, _ret_stats=False):
    x = np.asarray(x, np.float32)
    edge_src = np.asarray(edge_src).astype(np.int64)
    edge_dst = np.asarray(edge_dst).astype(np.int64)
    plan = _build_host_plan(x, edge_src, edge_dst, W1, b1, W2, b2)
    res = _run(plan, np.asarray(W1), np.asarray(b1), np.asarray(W2), np.asarray(b2),
               trace=_trace)
    N = plan["N"]
    out = np.zeros((N, D), np.float32)
    for c in range(NCORES):
        o = res.results[c]["out"]
        v = plan["valid"][c]
        out[plan["inv"][c][v]] = o[v]
    if _ret_stats:
        return out, res
    return out
